# revision 19
# baseline (speedup 1.0000x reference)
"""YOLO-style loss kernel for Trainium2, 8-core data-parallel (v3).

Strategy: shard the 16384 batch across 8 cores (2048 each = 100352 grid
cells). Each core streams its [cells, 30] fp32 pred/target arrays through
SBUF in 4 wide tiles (~6 MB/tile, 16 DMA queues).

v3 is tuned against the instruction cost model's DVE perf-mode rules:
tensor_tensor only reaches 2x when ALL operands are bf16 with a packed
(stride-1, count>=2) innermost dim, so the whole DVE dataflow is bf16 and
every access pattern is arranged to keep the innermost dim packed. Pair
arrays use a "k-layout" [diag(0,0) diag(1,1) off(0,1) off(1,0)] so the
xy-localization term can read the diagonal |dx| values packed. fp32->bf16
feeder ops (pairwise sums/width-diffs) that are stuck at 1x anyway are
offloaded to the GpSimd engine; Abs/Relu/Sqrt/Square run on Act.

Per-axis interval overlap avoids corner arithmetic:
  ow = (wp + wt)/2 - max(|cxp - cxt|, |wp - wt|/2),   cx = x/7
IoU = inter/union, union = areap_i + areat_j - inter, hardware reciprocal.
Responsible-box selection / last-write-wins conf targets match the
reference argmax semantics:
  g_j = iou(1,j) > iou(0,j), m_j = max_i iou(i,j)
  ct0 = m1 + g1*(m0-m1), ct1 = m0 - g1*(m0-m1)   (wrong cases resp-masked)
  resp_0 = obj*(1-g0*g1), resp_1 = obj*max(g0,g1)
The class term uses obj in {0,1}: sum(obj*d_c^2) = obj * sum_c(d_c^2) via
Act-square + packed bf16 add-tree (20 -> 5 -> 2 -> 1). Loss weights are
folded into masks (sqrt5, sqrt.5; xy reuses |dx|*2/7 via scale sqrt5*7/2).
One Act Square+accum reduces the 12-lane strip, one Copy+accum the class
sums; the host sums 8x[128, 2*NT] partials and divides by N.
"""

import math

import numpy as np
import concourse.bass as bass
import concourse.tile as tile
from concourse import mybir
from concourse.bass_utils import run_bass_kernel_spmd

F32 = mybir.dt.float32
BF16 = mybir.dt.bfloat16
Alu = mybir.AluOpType
Act = mybir.ActivationFunctionType

# problem constants (hardcoded per harness contract)
BATCH = 16384
S = 7
D = 30
N_CORES = 8
B_PER = BATCH // N_CORES            # 2048
K_CORE = B_PER * S * S              # 100352 cells/core
P = 128
CELLS_PER_PART = K_CORE // P        # 784
TILES = (98, 196, 196, 196, 98)     # per-tile cells/partition (head+tail taper)
NT = len(TILES)
CPP = max(TILES)                    # buffer sizing (tiles use subviews)
EPS = 1e-6
SQRT5 = math.sqrt(5.0)
SQRT_HALF = math.sqrt(0.5)
NGROUPS = 2                         # acc columns per tile: strip, class
SW = 12                             # strip: 0:2 contain | 2:4 noobj
                                    #        4:8 xy [d][b] | 8:12 wh [d][b]


def split_sync_waits(nc, max_attached=1):
    """This container's walrus build rejects >1 semaphore wait attached to an
    instruction. Hoist the extras into standalone EventSemaphore wait
    instructions (what raw-bass wait_ge emits), which it accepts."""
    n = 0
    for func in nc.m.functions:
        for bb in func.blocks:
            insts = list(bb.instructions)
            out = []
            changed = False
            for inst in insts:
                si = inst.sync_info
                if si is not None and len(si.on_wait) > max_attached:
                    waits = list(si.on_wait)
                    keep, hoist = waits[:max_attached], waits[max_attached:]
                    for k, w in enumerate(hoist):
                        wi = mybir.InstEventSemaphore(
                            name=f"{inst.name}-hw{k}", ins=[], outs=[]
                        )
                        wi.engine = inst.engine
                        wi.sync_info = mybir.SyncInfo(on_wait=[w], on_update=[])
                        nc.register_instruction(wi, overwrite=True)
                        out.append(wi)
                        n += 1
                    inst.sync_info = mybir.SyncInfo(
                        on_wait=keep, on_update=list(si.on_update)
                    )
                    changed = True
                out.append(inst)
            if changed:
                while len(bb.instructions):
                    bb.instructions.pop()
                for i in out:
                    bb.instructions.append(i)
    return n


def mkap(t_ap, off, dims):
    """AP into a [P, ...] tile/view: keep partition dim, custom free dims.
    dims = list of [stride_elems, count]."""
    return bass.AP(tensor=t_ap.tensor, offset=t_ap.offset + off,
                   ap=[list(t_ap.ap[0])] + [list(d) for d in dims])


def ch(t, c0, dims, cp=None):
    """Box-channel view of an io tile t ([P, CPP*30]): cell-major, channel c0,
    extra dims appended after the cell dim."""
    return mkap(t[:], c0, [[D, cp or CPP]] + dims)


def build_kernel(repeat=1, timing=False):
    nc = bass.Bass("TRN2")
    kind = "Internal" if timing else "ExternalInput"
    pred = nc.dram_tensor("pred", [K_CORE, D], F32, kind=kind)
    targ = nc.dram_tensor("targ", [K_CORE, D], F32, kind=kind)
    NTR = NT * repeat
    out = nc.dram_tensor("out", [P, NTR * NGROUPS], F32, kind="ExternalOutput")

    def dram_ap(t, cb, cp):
        a = t.ap()
        return bass.AP(tensor=a.tensor, offset=cb * P * D,
                       ap=[[cp * D, P], [1, cp * D]])

    with tile.TileContext(nc) as tc:
        with (
            tc.tile_pool(name="io", bufs=2) as io,
            tc.tile_pool(name="strip", bufs=2) as sp,
            tc.tile_pool(name="big", bufs=2) as big,
            tc.tile_pool(name="mid", bufs=2) as mid,
            tc.tile_pool(name="accp", bufs=1) as accp,
        ):
            acc = accp.tile([P, NTR * NGROUPS], F32)
            eps_t = accp.tile([P, 1], F32)
            zero_t = accp.tile([P, 1], F32)
            nc.vector.memset(eps_t[:], EPS)
            nc.vector.memset(zero_t[:], 0.0)

            pending = []  # deferred accumulates: (strip_ap, ss_ap, base)
            for rit in range(NTR):
                cp = TILES[rit % NT]
                cb = sum(TILES[:rit % NT])
                pt = io.tile([P, CPP * D], F32, tag="pt")
                tt = io.tile([P, CPP * D], F32, tag="tt")
                nc.sync.dma_start(out=mkap(pt[:], 0, [[1, cp * D]]),
                                  in_=dram_ap(pred, cb, cp))
                nc.sync.dma_start(out=mkap(tt[:], 0, [[1, cp * D]]),
                                  in_=dram_ap(targ, cb, cp))

                strip = sp.tile([P, CPP, SW], BF16, tag="strip")

                # ---- pairwise arrays, k-layout [P, C, 2d, 4k], all bf16 ----
                # k = [(j0,i0), (j1,i1), (j0,i1), (j1,i0)]; pred i-pattern
                # (0,1,1,0) -> diag stride +5 / off stride -5 from box1;
                # targ j-pattern (0,1,0,1) -> stride +5 both halves.
                sxy = big.tile([P, CPP, 8], BF16, tag="sxy")     # pair sums
                aaw = big.tile([P, CPP, 16], BF16, tag="aaw")    # 0:8 xy, 8:16 wh
                dcls = big.tile([P, CPP, 4, 5], BF16, tag="dcls")
                for half, istr, ioff in ((0, 5, 0), (2, -5, 5)):
                    # s = wp_i + wt_j (Pool), a = xp_i - xt_j (DVE),
                    # dw = wp_i - wt_j (Pool)
                    s_out = mkap(sxy[:], half, [[8, cp], [4, 2], [1, 2]])
                    nc.gpsimd.tensor_tensor(
                        out=s_out,
                        in0=ch(pt, 2 + ioff, [[1, 2], [istr, 2]], cp),
                        in1=ch(tt, 2, [[1, 2], [5, 2]], cp), op=Alu.add)
                    a_out = mkap(aaw[:], half, [[16, cp], [4, 2], [1, 2]])
                    nc.vector.tensor_tensor(
                        out=a_out,
                        in0=ch(pt, 0 + ioff, [[1, 2], [istr, 2]], cp),
                        in1=ch(tt, 0, [[1, 2], [5, 2]], cp), op=Alu.subtract)
                    w_out = mkap(aaw[:], 8 + half, [[16, cp], [4, 2], [1, 2]])
                    nc.gpsimd.tensor_tensor(
                        out=w_out,
                        in0=ch(pt, 2 + ioff, [[1, 2], [istr, 2]], cp),
                        in1=ch(tt, 2, [[1, 2], [5, 2]], cp), op=Alu.subtract)

                # class diffs early (only needs io tiles); Act squares them
                # while the DVE iou chain runs. Last 5 channels on Pool.
                dcls_f = mkap(dcls[:], 0, [[1, cp * 20]])
                dcls_15 = mkap(dcls[:], 0, [[20, cp], [1, 15]])
                dcls_5 = mkap(dcls[:], 15, [[20, cp], [1, 5]])
                nc.vector.tensor_tensor(out=dcls_15, in0=ch(pt, 10, [[1, 15]], cp),
                                        in1=ch(tt, 10, [[1, 15]], cp),
                                        op=Alu.subtract)
                nc.gpsimd.tensor_tensor(out=dcls_5, in0=ch(pt, 25, [[1, 5]], cp),
                                        in1=ch(tt, 25, [[1, 5]], cp),
                                        op=Alu.subtract)

                # Act: |a|*2/7, |dw| (in place), sqrt(wh+eps), square(dcls)
                aa = mkap(aaw[:], 0, [[16, cp], [1, 8]])
                ww = mkap(aaw[:], 8, [[16, cp], [1, 8]])
                nc.scalar.activation(out=aa, in_=aa, func=Act.Abs,
                                     bias=zero_t[:], scale=2.0 / S)
                nc.scalar.activation(out=ww, in_=ww, func=Act.Abs,
                                     bias=zero_t[:], scale=1.0)
                sqt = big.tile([P, CPP, 4], BF16, tag="sqt")     # [d][b]
                st_whT = mkap(strip[:], 8, [[SW, cp], [1, 2], [2, 2]])
                nc.scalar.activation(out=st_whT, in_=ch(pt, 2, [[5, 2], [1, 2]], cp),
                                     func=Act.Sqrt, bias=eps_t[:], scale=1.0)
                sqt_T = mkap(sqt[:], 0, [[4, cp], [1, 2], [2, 2]])
                nc.scalar.activation(out=sqt_T, in_=ch(tt, 2, [[5, 2], [1, 2]], cp),
                                     func=Act.Sqrt, bias=eps_t[:], scale=1.0)
                nc.scalar.activation(out=dcls_f, in_=dcls_f, func=Act.Square,
                                     scale=1.0)

                # previous tile's accumulates, after this tile's Act front
                # ops so they don't head-of-line block the abs/sqrt feeds
                while pending:
                    p_strip, p_ss, p_base = pending.pop()
                    nc.scalar.activation(out=p_strip, in_=p_strip,
                                         func=Act.Square, scale=1.0,
                                         accum_out=acc[:, p_base:p_base + 1])
                    nc.scalar.activation(out=p_ss, in_=p_ss, func=Act.Copy,
                                         scale=1.0, bias=0.0,
                                         accum_out=acc[:, p_base + 1:p_base + 2])

                # hm = max(|a|2/7, |dw|) into the dw half; q = s - hm;
                # oc = relu(q)*0.5 via tensor_scalar. All bf16 packed.
                nc.vector.tensor_tensor(out=ww, in0=aa, in1=ww, op=Alu.max)
                sxy_f = mkap(sxy[:], 0, [[1, cp * 8]])
                nc.vector.tensor_tensor(out=sxy_f, in0=sxy_f, in1=ww,
                                        op=Alu.subtract)
                oc = big.tile([P, CPP, 8], BF16, tag="oc")
                oc_f = mkap(oc[:], 0, [[1, cp * 8]])
                nc.vector.tensor_scalar(out=oc_f, in0=sxy_f, scalar1=0.0,
                                        scalar2=0.5, op0=Alu.max, op1=Alu.mult)

                # inter[k] = ow_x[k] * ow_y[k] (bf16 2x)
                inter = mid.tile([P, CPP, 4], BF16, tag="inter")
                nc.vector.tensor_tensor(out=mkap(inter[:], 0, [[1, cp * 4]]),
                                        in0=mkap(oc[:], 0, [[8, cp], [1, 4]]),
                                        in1=mkap(oc[:], 4, [[8, cp], [1, 4]]),
                                        op=Alu.mult)

                # areas (Pool, bf16), pairwise sums in k-layout (Pool),
                # union = asum - inter (DVE 2x), 1/union -> f32 (DVE)
                areap = mid.tile([P, CPP, 2], BF16, tag="areap")
                areat = mid.tile([P, CPP, 2], BF16, tag="areat")
                nc.gpsimd.tensor_tensor(out=mkap(areap[:], 0, [[1, cp * 2]]),
                                        in0=ch(pt, 2, [[5, 2]], cp),
                                        in1=ch(pt, 3, [[5, 2]], cp), op=Alu.mult)
                nc.gpsimd.tensor_tensor(out=mkap(areat[:], 0, [[1, cp * 2]]),
                                        in0=ch(tt, 2, [[5, 2]], cp),
                                        in1=ch(tt, 3, [[5, 2]], cp), op=Alu.mult)
                uni = mid.tile([P, CPP, 4], BF16, tag="uni")
                nc.gpsimd.tensor_tensor(
                    out=mkap(uni[:], 0, [[4, cp], [1, 2]]),
                    in0=mkap(areap[:], 0, [[2, cp], [1, 2]]),
                    in1=mkap(areat[:], 0, [[2, cp], [1, 2]]), op=Alu.add)
                nc.gpsimd.tensor_tensor(
                    out=mkap(uni[:], 2, [[4, cp], [1, 2]]),
                    in0=mkap(areap[:], 1, [[2, cp], [-1, 2]]),
                    in1=mkap(areat[:], 0, [[2, cp], [1, 2]]), op=Alu.add)
                uni_f = mkap(uni[:], 0, [[1, cp * 4]])
                inter_f = mkap(inter[:], 0, [[1, cp * 4]])
                nc.vector.tensor_tensor(out=uni_f, in0=uni_f, in1=inter_f,
                                        op=Alu.subtract)
                run = mid.tile([P, CPP, 4], F32, tag="run")
                run_f = mkap(run[:], 0, [[1, cp * 4]])
                nc.vector.reciprocal(out=run_f, in_=uni_f)

                # iou written transposed to [i][j] (lane = 2i + j) so m/g
                # read packed over j: diag k(0,1)->(0,3), off k(2,3)->(2,1)
                iou = mid.tile([P, CPP, 4], BF16, tag="iou")
                nc.vector.tensor_tensor(
                    out=mkap(iou[:], 0, [[4, cp], [3, 2]]),
                    in0=mkap(inter[:], 0, [[4, cp], [1, 2]]),
                    in1=mkap(run[:], 0, [[4, cp], [1, 2]]), op=Alu.mult)
                nc.vector.tensor_tensor(
                    out=mkap(iou[:], 2, [[4, cp], [-1, 2]]),
                    in0=mkap(inter[:], 2, [[4, cp], [1, 2]]),
                    in1=mkap(run[:], 2, [[4, cp], [1, 2]]), op=Alu.mult)
                m = mid.tile([P, CPP, 2], BF16, tag="m")
                g = mid.tile([P, CPP, 2], BF16, tag="g")
                iou_i0 = mkap(iou[:], 0, [[4, cp], [1, 2]])
                iou_i1 = mkap(iou[:], 2, [[4, cp], [1, 2]])
                m_ = mkap(m[:], 0, [[1, cp * 2]])
                g_ = mkap(g[:], 0, [[1, cp * 2]])
                nc.vector.tensor_tensor(out=mkap(m[:], 0, [[2, cp], [1, 2]]),
                                        in0=iou_i0, in1=iou_i1, op=Alu.max)
                nc.vector.tensor_tensor(out=mkap(g[:], 0, [[2, cp], [1, 2]]),
                                        in0=iou_i1, in1=iou_i0, op=Alu.is_gt)

                # class add-tree 20 -> 5 -> 2(+1) -> 1 (packed bf16), after
                # m/g so it fills DVE idle while Pool/Act catch up
                tA = big.tile([P, CPP, 5], BF16, tag="tA")
                tB = big.tile([P, CPP, 5], BF16, tag="tB")
                dg = lambda k: mkap(dcls[:], 5 * k, [[20, cp], [1, 5]])
                tA_ = mkap(tA[:], 0, [[5, cp], [1, 5]])
                tB_ = mkap(tB[:], 0, [[5, cp], [1, 5]])
                nc.vector.tensor_tensor(out=tA_, in0=dg(0), in1=dg(1), op=Alu.add)
                nc.vector.tensor_tensor(out=tB_, in0=dg(2), in1=dg(3), op=Alu.add)
                nc.vector.tensor_tensor(out=tA_, in0=tA_, in1=tB_, op=Alu.add)
                f1 = big.tile([P, CPP, 2], BF16, tag="f1")
                nc.vector.tensor_tensor(out=mkap(f1[:], 0, [[2, cp], [1, 2]]),
                                        in0=mkap(tA[:], 0, [[5, cp], [1, 2]]),
                                        in1=mkap(tA[:], 2, [[5, cp], [1, 2]]),
                                        op=Alu.add)
                ss = big.tile([P, CPP], BF16, tag="ss")
                ss_ = mkap(ss[:], 0, [[1, cp]])
                nc.vector.tensor_tensor(out=ss_,
                                        in0=mkap(f1[:], 0, [[2, cp]]),
                                        in1=mkap(f1[:], 1, [[2, cp]]),
                                        op=Alu.add)
                nc.vector.tensor_tensor(out=ss_, in0=ss_,
                                        in1=mkap(tA[:], 4, [[5, cp]]),
                                        op=Alu.add)

                # masks: obj/noo on Pool (width 1 and 2)
                obj1 = mid.tile([P, CPP], BF16, tag="obj1")
                obj2 = mid.tile([P, CPP, 2], BF16, tag="obj2")
                noo2 = mid.tile([P, CPP, 2], BF16, tag="noo2")
                t4b = ch(tt, 4, [[0, 2]], cp)
                obj1_ = mkap(obj1[:], 0, [[1, cp]])
                obj2_ = mkap(obj2[:], 0, [[1, cp * 2]])
                noo2_ = mkap(noo2[:], 0, [[1, cp * 2]])
                nc.gpsimd.tensor_scalar(out=obj1_, in0=ch(tt, 4, [[1, 1]], cp),
                                        scalar1=0.0, scalar2=None, op0=Alu.is_gt)
                nc.gpsimd.tensor_scalar(out=obj2_, in0=t4b, scalar1=0.0,
                                        scalar2=None, op0=Alu.is_gt)
                nc.gpsimd.tensor_scalar(out=noo2_, in0=t4b, scalar1=0.0,
                                        scalar2=None, op0=Alu.is_le)
                nc.gpsimd.tensor_scalar(out=noo2_, in0=noo2_,
                                        scalar1=SQRT_HALF, scalar2=None,
                                        op0=Alu.mult)

                # conf targets + responsibility masks
                m0, m1 = mkap(m[:], 0, [[2, cp], [1, 1]]), mkap(m[:], 1, [[2, cp], [1, 1]])
                g0, g1 = mkap(g[:], 0, [[2, cp], [1, 1]]), mkap(g[:], 1, [[2, cp], [1, 1]])
                dm = mid.tile([P, CPP, 1], BF16, tag="dm")
                gdm = mid.tile([P, CPP, 1], BF16, tag="gdm")
                ct = mid.tile([P, CPP, 2], BF16, tag="ct")
                dm_ = mkap(dm[:], 0, [[1, cp], [0, 1]])
                gdm_ = mkap(gdm[:], 0, [[1, cp], [0, 1]])
                nc.vector.tensor_tensor(out=dm_, in0=m0, in1=m1, op=Alu.subtract)
                nc.vector.tensor_tensor(out=gdm_, in0=g1, in1=dm_, op=Alu.mult)
                nc.vector.tensor_tensor(out=mkap(ct[:], 0, [[2, cp], [1, 1]]),
                                        in0=m1, in1=gdm_, op=Alu.add)
                nc.vector.tensor_tensor(out=mkap(ct[:], 1, [[2, cp], [1, 1]]),
                                        in0=m0, in1=gdm_, op=Alu.subtract)
                gmin = mid.tile([P, CPP, 1], BF16, tag="gmin")
                rr = mid.tile([P, CPP, 2], BF16, tag="rr")
                gmin_ = mkap(gmin[:], 0, [[1, cp], [0, 1]])
                nc.vector.tensor_tensor(out=gmin_, in0=g0, in1=g1, op=Alu.mult)
                nc.vector.tensor_scalar(out=mkap(rr[:], 0, [[2, cp], [1, 1]]),
                                        in0=gmin_, scalar1=-1.0, scalar2=1.0,
                                        op0=Alu.mult, op1=Alu.add)
                nc.vector.tensor_tensor(out=mkap(rr[:], 1, [[2, cp], [1, 1]]),
                                        in0=g0, in1=g1, op=Alu.max)
                rm = mid.tile([P, CPP, 2], BF16, tag="rm")
                rm5 = mid.tile([P, CPP, 2], BF16, tag="rm5")
                rm5x = mid.tile([P, CPP, 2], BF16, tag="rm5x")
                rm_ = mkap(rm[:], 0, [[1, cp * 2]])
                nc.vector.tensor_tensor(out=rm_, in0=mkap(rr[:], 0, [[1, cp * 2]]),
                                        in1=obj2_, op=Alu.mult)
                rm5_ = mkap(rm5[:], 0, [[1, cp * 2]])
                rm5x_ = mkap(rm5x[:], 0, [[1, cp * 2]])
                nc.vector.tensor_scalar(out=rm5_, in0=rm_, scalar1=SQRT5,
                                        scalar2=None, op0=Alu.mult)
                nc.vector.tensor_scalar(out=rm5x_, in0=rm_,
                                        scalar1=SQRT5 * S / 2.0,
                                        scalar2=None, op0=Alu.mult)

                # class mask by obj + strip writers
                nc.vector.tensor_tensor(out=ss_, in0=ss_, in1=obj1_,
                                        op=Alu.mult)

                # contain: (pc - ct) * rm -> strip[0:2]
                st_e = mkap(strip[:], 0, [[SW, cp], [1, 2]])
                nc.vector.tensor_tensor(out=st_e, in0=ch(pt, 4, [[5, 2]], cp),
                                        in1=mkap(ct[:], 0, [[1, cp * 2]])
                                        .rearrange("p (c k) -> p c k", k=2),
                                        op=Alu.subtract)
                nc.vector.tensor_tensor(out=st_e, in0=st_e,
                                        in1=mkap(rm[:], 0, [[2, cp], [1, 2]]),
                                        op=Alu.mult)

                # noobj: (p49 - t49) * noo*sqrt(.5) -> strip[2:4] (Pool)
                st_n = mkap(strip[:], 2, [[SW, cp], [1, 2]])
                nc.gpsimd.tensor_tensor(out=st_n, in0=ch(pt, 4, [[5, 2]], cp),
                                        in1=ch(tt, 4, [[5, 2]], cp),
                                        op=Alu.subtract)
                nc.gpsimd.tensor_tensor(out=st_n, in0=st_n,
                                        in1=mkap(noo2[:], 0, [[2, cp], [1, 2]]),
                                        op=Alu.mult)

                # loc xy: packed diag |a|*2/7 times rm5x -> strip[4:8] [d][b]
                adiag = mkap(aaw[:], 0, [[16, cp], [4, 2], [1, 2]])
                st_xy = mkap(strip[:], 4, [[SW, cp], [2, 2], [1, 2]])
                rm5x_b = mkap(rm5x[:], 0, [[2, cp], [0, 2], [1, 2]])
                nc.vector.tensor_tensor(out=st_xy, in0=adiag, in1=rm5x_b,
                                        op=Alu.mult)

                # loc wh: strip[8:12] -= sqt, then times rm5 ([d][b])
                st_wh = mkap(strip[:], 8, [[SW, cp], [1, 4]])
                nc.vector.tensor_tensor(out=st_wh, in0=st_wh,
                                        in1=mkap(sqt[:], 0, [[4, cp], [1, 4]]),
                                        op=Alu.subtract)
                st_whdb = mkap(strip[:], 8, [[SW, cp], [2, 2], [1, 2]])
                rm5_b = mkap(rm5[:], 0, [[2, cp], [0, 2], [1, 2]])
                nc.vector.tensor_tensor(out=st_whdb, in0=st_whdb, in1=rm5_b,
                                        op=Alu.mult)

                # defer accumulation into the next iteration's Act queue
                base = rit * NGROUPS
                s_flat = mkap(strip[:], 0, [[1, cp * SW]])
                pending.append((s_flat, ss_, base))

            while pending:
                p_strip, p_ss, p_base = pending.pop()
                nc.scalar.activation(out=p_strip, in_=p_strip, func=Act.Square,
                                     scale=1.0, accum_out=acc[:, p_base:p_base + 1])
                nc.scalar.activation(out=p_ss, in_=p_ss, func=Act.Copy,
                                     scale=1.0, bias=0.0,
                                     accum_out=acc[:, p_base + 1:p_base + 2])

            nc.sync.dma_start(out=out[:], in_=acc[:])

    split_sync_waits(nc)
    return nc


_NC_CACHE = None


def kernel(pred_tensor: np.ndarray, target_tensor: np.ndarray) -> np.ndarray:
    global _NC_CACHE
    if _NC_CACHE is None:
        _NC_CACHE = build_kernel()
    nc = _NC_CACHE

    p = np.ascontiguousarray(pred_tensor, dtype=np.float32).reshape(N_CORES, K_CORE, D)
    t = np.ascontiguousarray(target_tensor, dtype=np.float32).reshape(N_CORES, K_CORE, D)
    in_maps = [{"pred": p[i], "targ": t[i]} for i in range(N_CORES)]
    res = run_bass_kernel_spmd(nc, in_maps, core_ids=list(range(N_CORES)))
    total = 0.0
    for i in range(N_CORES):
        total += res.results[i]["out"].astype(np.float64).sum()
    return np.float32(total / BATCH)


# revision 21
# speedup vs baseline: 2.0264x; 2.0264x over previous
"""YOLO-style loss kernel for Trainium2, 8-core data-parallel (v3).

Strategy: shard the 16384 batch across 8 cores (2048 each = 100352 grid
cells). Each core streams its [cells, 30] fp32 pred/target arrays through
SBUF in 4 wide tiles (~6 MB/tile, 16 DMA queues).

v3 is tuned against the instruction cost model's DVE perf-mode rules:
tensor_tensor only reaches 2x when ALL operands are bf16 with a packed
(stride-1, count>=2) innermost dim, so the whole DVE dataflow is bf16 and
every access pattern is arranged to keep the innermost dim packed. Pair
arrays use a "k-layout" [diag(0,0) diag(1,1) off(0,1) off(1,0)] so the
xy-localization term can read the diagonal |dx| values packed. fp32->bf16
feeder ops (pairwise sums/width-diffs) that are stuck at 1x anyway are
offloaded to the GpSimd engine; Abs/Relu/Sqrt/Square run on Act.

Per-axis interval overlap avoids corner arithmetic:
  ow = (wp + wt)/2 - max(|cxp - cxt|, |wp - wt|/2),   cx = x/7
IoU = inter/union, union = areap_i + areat_j - inter, hardware reciprocal.
Responsible-box selection / last-write-wins conf targets match the
reference argmax semantics:
  g_j = iou(1,j) > iou(0,j), m_j = max_i iou(i,j)
  ct0 = m1 + g1*(m0-m1), ct1 = m0 - g1*(m0-m1)   (wrong cases resp-masked)
  resp_0 = obj*(1-g0*g1), resp_1 = obj*max(g0,g1)
The class term uses obj in {0,1}: sum(obj*d_c^2) = obj * sum_c(d_c^2) via
Act-square + packed bf16 add-tree (20 -> 5 -> 2 -> 1). Loss weights are
folded into masks (sqrt5, sqrt.5; xy reuses |dx|*2/7 via scale sqrt5*7/2).
One Act Square+accum reduces the 12-lane strip, one Copy+accum the class
sums; the host sums 8x[128, 2*NT] partials and divides by N.
"""

import math

import numpy as np
import concourse.bass as bass
import concourse.tile as tile
from concourse import mybir
from concourse.bass_utils import run_bass_kernel_spmd

F32 = mybir.dt.float32
BF16 = mybir.dt.bfloat16
Alu = mybir.AluOpType
Act = mybir.ActivationFunctionType

# problem constants (hardcoded per harness contract)
BATCH = 16384
S = 7
D = 30
N_CORES = 8
B_PER = BATCH // N_CORES            # 2048
K_CORE = B_PER * S * S              # 100352 cells/core
P = 128
CELLS_PER_PART = K_CORE // P        # 784
TILES = (98, 196, 196, 196, 98)     # per-tile cells/partition (head+tail taper)
NT = len(TILES)
CPP = max(TILES)                    # buffer sizing (tiles use subviews)
EPS = 1e-6
SQRT5 = math.sqrt(5.0)
SQRT_HALF = math.sqrt(0.5)
NGROUPS = 2                         # acc columns per tile: strip, class
SW = 12                             # strip: 0:2 contain | 2:4 noobj
                                    #        4:8 xy [d][b] | 8:12 wh [d][b]


def split_sync_waits(nc, max_attached=1):
    """This container's walrus build rejects >1 semaphore wait attached to an
    instruction. Hoist the extras into standalone EventSemaphore wait
    instructions (what raw-bass wait_ge emits), which it accepts."""
    n = 0
    for func in nc.m.functions:
        for bb in func.blocks:
            insts = list(bb.instructions)
            out = []
            changed = False
            for inst in insts:
                si = inst.sync_info
                if si is not None and len(si.on_wait) > max_attached:
                    waits = list(si.on_wait)
                    keep, hoist = waits[:max_attached], waits[max_attached:]
                    for k, w in enumerate(hoist):
                        wi = mybir.InstEventSemaphore(
                            name=f"{inst.name}-hw{k}", ins=[], outs=[]
                        )
                        wi.engine = inst.engine
                        wi.sync_info = mybir.SyncInfo(on_wait=[w], on_update=[])
                        nc.register_instruction(wi, overwrite=True)
                        out.append(wi)
                        n += 1
                    inst.sync_info = mybir.SyncInfo(
                        on_wait=keep, on_update=list(si.on_update)
                    )
                    changed = True
                out.append(inst)
            if changed:
                while len(bb.instructions):
                    bb.instructions.pop()
                for i in out:
                    bb.instructions.append(i)
    return n


def mkap(t_ap, off, dims):
    """AP into a [P, ...] tile/view: keep partition dim, custom free dims.
    dims = list of [stride_elems, count]."""
    return bass.AP(tensor=t_ap.tensor, offset=t_ap.offset + off,
                   ap=[list(t_ap.ap[0])] + [list(d) for d in dims])


def ch(t, c0, dims, cp=None):
    """Box-channel view of an io tile t ([P, CPP*30]): cell-major, channel c0,
    extra dims appended after the cell dim."""
    return mkap(t[:], c0, [[D, cp or CPP]] + dims)


def build_kernel(repeat=1, timing=False):
    nc = bass.Bass("TRN2")
    kind = "Internal" if timing else "ExternalInput"
    pred = nc.dram_tensor("pred", [K_CORE, D], F32, kind=kind)
    targ = nc.dram_tensor("targ", [K_CORE, D], F32, kind=kind)
    NTR = NT * repeat
    out = nc.dram_tensor("out", [P, NTR * NGROUPS], F32, kind="ExternalOutput")

    def dram_ap(t, cb, cp):
        a = t.ap()
        return bass.AP(tensor=a.tensor, offset=cb * P * D,
                       ap=[[cp * D, P], [1, cp * D]])

    with tile.TileContext(nc) as tc:
        with (
            tc.tile_pool(name="io", bufs=2) as io,
            tc.tile_pool(name="strip", bufs=2) as sp,
            tc.tile_pool(name="big", bufs=2) as big,
            tc.tile_pool(name="mid", bufs=2) as mid,
            tc.tile_pool(name="accp", bufs=1) as accp,
        ):
            acc = accp.tile([P, NTR * NGROUPS], F32)
            eps_t = accp.tile([P, 1], F32)
            zero_t = accp.tile([P, 1], F32)
            nc.vector.memset(eps_t[:], EPS)
            nc.vector.memset(zero_t[:], 0.0)

            pending = []  # deferred accumulates: (strip_ap, ss_ap, base)
            for rit in range(NTR):
                cp = TILES[rit % NT]
                cb = sum(TILES[:rit % NT])
                pt = io.tile([P, CPP * D], F32, tag="pt")
                tt = io.tile([P, CPP * D], F32, tag="tt")
                nc.sync.dma_start(out=mkap(pt[:], 0, [[1, cp * D]]),
                                  in_=dram_ap(pred, cb, cp))
                nc.sync.dma_start(out=mkap(tt[:], 0, [[1, cp * D]]),
                                  in_=dram_ap(targ, cb, cp))

                strip = sp.tile([P, CPP, SW], BF16, tag="strip")

                # ---- pairwise arrays, k-layout [P, C, 2d, 4k], all bf16 ----
                # k = [(j0,i0), (j1,i1), (j0,i1), (j1,i0)]; pred i-pattern
                # (0,1,1,0) -> diag stride +5 / off stride -5 from box1;
                # targ j-pattern (0,1,0,1) -> stride +5 both halves.
                sxy = big.tile([P, CPP, 8], BF16, tag="sxy")     # pair sums
                axy = big.tile([P, CPP, 8], BF16, tag="axy")     # xy diffs
                awh = big.tile([P, CPP, 8], BF16, tag="awh")     # wh diffs
                dcls = big.tile([P, CPP, 4, 5], BF16, tag="dcls")
                for half, istr, ioff in ((0, 5, 0), (2, -5, 5)):
                    # s = wp_i + wt_j (Pool), dw = wp_i - wt_j (Pool;
                    # negative strides are fine on GpSimd)
                    s_out = mkap(sxy[:], half, [[8, cp], [4, 2], [1, 2]])
                    nc.gpsimd.tensor_tensor(
                        out=s_out,
                        in0=ch(pt, 2 + ioff, [[1, 2], [istr, 2]], cp),
                        in1=ch(tt, 2, [[1, 2], [5, 2]], cp), op=Alu.add)
                    w_out = mkap(awh[:], half, [[8, cp], [4, 2], [1, 2]])
                    nc.gpsimd.tensor_tensor(
                        out=w_out,
                        in0=ch(pt, 2 + ioff, [[1, 2], [istr, 2]], cp),
                        in1=ch(tt, 2, [[1, 2], [5, 2]], cp), op=Alu.subtract)
                # a = xp_i - xt_j (DVE): diag in one op, off-diag as two
                # singleton ops so no negative strides hit the DVE
                nc.vector.tensor_tensor(
                    out=mkap(axy[:], 0, [[8, cp], [4, 2], [1, 2]]),
                    in0=ch(pt, 0, [[1, 2], [5, 2]], cp),
                    in1=ch(tt, 0, [[1, 2], [5, 2]], cp), op=Alu.subtract)
                nc.vector.tensor_tensor(
                    out=mkap(axy[:], 2, [[8, cp], [4, 2], [1, 1]]),
                    in0=ch(pt, 5, [[1, 2], [1, 1]], cp),
                    in1=ch(tt, 0, [[1, 2], [1, 1]], cp), op=Alu.subtract)
                nc.vector.tensor_tensor(
                    out=mkap(axy[:], 3, [[8, cp], [4, 2], [1, 1]]),
                    in0=ch(pt, 0, [[1, 2], [1, 1]], cp),
                    in1=ch(tt, 5, [[1, 2], [1, 1]], cp), op=Alu.subtract)

                # class diffs early (only needs io tiles); Act squares them
                # while the DVE iou chain runs. Last 5 channels on Pool.
                dcls_f = mkap(dcls[:], 0, [[1, cp * 20]])
                dcls_15 = mkap(dcls[:], 0, [[20, cp], [1, 15]])
                dcls_5 = mkap(dcls[:], 15, [[20, cp], [1, 5]])
                nc.vector.tensor_tensor(out=dcls_15, in0=ch(pt, 10, [[1, 15]], cp),
                                        in1=ch(tt, 10, [[1, 15]], cp),
                                        op=Alu.subtract)
                nc.gpsimd.tensor_tensor(out=dcls_5, in0=ch(pt, 25, [[1, 5]], cp),
                                        in1=ch(tt, 25, [[1, 5]], cp),
                                        op=Alu.subtract)

                # Act: |a|*2/7, |dw| (in place, contiguous), sqrt(wh+eps)
                # (contiguous [b][d] writes), square(dcls)
                aa = mkap(axy[:], 0, [[1, cp * 8]])
                ww = mkap(awh[:], 0, [[1, cp * 8]])
                nc.scalar.activation(out=aa, in_=aa, func=Act.Abs,
                                     bias=zero_t[:], scale=2.0 / S)
                nc.scalar.activation(out=ww, in_=ww, func=Act.Abs,
                                     bias=zero_t[:], scale=1.0)
                sqt = big.tile([P, CPP, 4], BF16, tag="sqt")     # [b][d]
                st_whC = mkap(strip[:], 8, [[SW, cp], [1, 4]])
                nc.scalar.activation(out=st_whC, in_=ch(pt, 2, [[5, 2], [1, 2]], cp),
                                     func=Act.Sqrt, bias=eps_t[:], scale=1.0)
                nc.scalar.activation(out=mkap(sqt[:], 0, [[1, cp * 4]]),
                                     in_=ch(tt, 2, [[5, 2], [1, 2]], cp),
                                     func=Act.Sqrt, bias=eps_t[:], scale=1.0)
                nc.scalar.activation(out=dcls_f, in_=dcls_f, func=Act.Square,
                                     scale=1.0)

                # previous tile's accumulates, after this tile's Act front
                # ops so they don't head-of-line block the abs/sqrt feeds
                while pending:
                    p_strip, p_ss, p_base = pending.pop()
                    nc.scalar.activation(out=p_strip, in_=p_strip,
                                         func=Act.Square, scale=1.0,
                                         accum_out=acc[:, p_base:p_base + 1])
                    nc.scalar.activation(out=p_ss, in_=p_ss, func=Act.Copy,
                                         scale=1.0, bias=0.0,
                                         accum_out=acc[:, p_base + 1:p_base + 2])

                # hm = max(|a|2/7, |dw|) into the dw half; q = s - hm;
                # oc = relu(q)*0.5 via tensor_scalar. All bf16 packed.
                nc.vector.tensor_tensor(out=ww, in0=aa, in1=ww, op=Alu.max)
                sxy_f = mkap(sxy[:], 0, [[1, cp * 8]])
                nc.vector.tensor_tensor(out=sxy_f, in0=sxy_f, in1=ww,
                                        op=Alu.subtract)
                del aa, ww
                oc = big.tile([P, CPP, 8], BF16, tag="oc")
                oc_f = mkap(oc[:], 0, [[1, cp * 8]])
                nc.vector.tensor_scalar(out=oc_f, in0=sxy_f, scalar1=0.0,
                                        scalar2=0.5, op0=Alu.max, op1=Alu.mult)

                # inter[k] = ow_x[k] * ow_y[k] (bf16 2x)
                inter = mid.tile([P, CPP, 4], BF16, tag="inter")
                nc.vector.tensor_tensor(out=mkap(inter[:], 0, [[1, cp * 4]]),
                                        in0=mkap(oc[:], 0, [[8, cp], [1, 4]]),
                                        in1=mkap(oc[:], 4, [[8, cp], [1, 4]]),
                                        op=Alu.mult)

                # areas (Pool, bf16), pairwise sums in k-layout (Pool),
                # union = asum - inter (DVE 2x), 1/union -> f32 (DVE)
                areap = mid.tile([P, CPP, 2], BF16, tag="areap")
                areat = mid.tile([P, CPP, 2], BF16, tag="areat")
                nc.gpsimd.tensor_tensor(out=mkap(areap[:], 0, [[1, cp * 2]]),
                                        in0=ch(pt, 2, [[5, 2]], cp),
                                        in1=ch(pt, 3, [[5, 2]], cp), op=Alu.mult)
                nc.gpsimd.tensor_tensor(out=mkap(areat[:], 0, [[1, cp * 2]]),
                                        in0=ch(tt, 2, [[5, 2]], cp),
                                        in1=ch(tt, 3, [[5, 2]], cp), op=Alu.mult)
                uni = mid.tile([P, CPP, 4], BF16, tag="uni")
                nc.gpsimd.tensor_tensor(
                    out=mkap(uni[:], 0, [[4, cp], [1, 2]]),
                    in0=mkap(areap[:], 0, [[2, cp], [1, 2]]),
                    in1=mkap(areat[:], 0, [[2, cp], [1, 2]]), op=Alu.add)
                nc.gpsimd.tensor_tensor(
                    out=mkap(uni[:], 2, [[4, cp], [1, 2]]),
                    in0=mkap(areap[:], 1, [[2, cp], [-1, 2]]),
                    in1=mkap(areat[:], 0, [[2, cp], [1, 2]]), op=Alu.add)
                uni_f = mkap(uni[:], 0, [[1, cp * 4]])
                inter_f = mkap(inter[:], 0, [[1, cp * 4]])
                nc.vector.tensor_tensor(out=uni_f, in0=uni_f, in1=inter_f,
                                        op=Alu.subtract)
                run = mid.tile([P, CPP, 4], F32, tag="run")
                run_f = mkap(run[:], 0, [[1, cp * 4]])
                nc.vector.reciprocal(out=run_f, in_=uni_f)

                # iou written transposed to [i][j] (lane = 2i + j) so m/g
                # read packed over j: diag k(0,1)->(0,3), off k(2,3)->(2,1)
                iou = mid.tile([P, CPP, 4], BF16, tag="iou")
                nc.vector.tensor_tensor(
                    out=mkap(iou[:], 0, [[4, cp], [3, 2]]),
                    in0=mkap(inter[:], 0, [[4, cp], [1, 2]]),
                    in1=mkap(run[:], 0, [[4, cp], [1, 2]]), op=Alu.mult)
                nc.vector.tensor_tensor(
                    out=mkap(iou[:], 2, [[4, cp], [1, 1]]),
                    in0=mkap(inter[:], 2, [[4, cp], [1, 1]]),
                    in1=mkap(run[:], 2, [[4, cp], [1, 1]]), op=Alu.mult)
                nc.vector.tensor_tensor(
                    out=mkap(iou[:], 1, [[4, cp], [1, 1]]),
                    in0=mkap(inter[:], 3, [[4, cp], [1, 1]]),
                    in1=mkap(run[:], 3, [[4, cp], [1, 1]]), op=Alu.mult)
                m = mid.tile([P, CPP, 2], BF16, tag="m")
                g = mid.tile([P, CPP, 2], BF16, tag="g")
                iou_i0 = mkap(iou[:], 0, [[4, cp], [1, 2]])
                iou_i1 = mkap(iou[:], 2, [[4, cp], [1, 2]])
                m_ = mkap(m[:], 0, [[1, cp * 2]])
                g_ = mkap(g[:], 0, [[1, cp * 2]])
                nc.vector.tensor_tensor(out=mkap(m[:], 0, [[2, cp], [1, 2]]),
                                        in0=iou_i0, in1=iou_i1, op=Alu.max)
                nc.vector.tensor_tensor(out=mkap(g[:], 0, [[2, cp], [1, 2]]),
                                        in0=iou_i1, in1=iou_i0, op=Alu.is_gt)

                # class add-tree 20 -> 5 -> 2(+1) -> 1 (packed bf16), after
                # m/g so it fills DVE idle while Pool/Act catch up
                tA = big.tile([P, CPP, 5], BF16, tag="tA")
                tB = big.tile([P, CPP, 5], BF16, tag="tB")
                dg = lambda k: mkap(dcls[:], 5 * k, [[20, cp], [1, 5]])
                tA_ = mkap(tA[:], 0, [[5, cp], [1, 5]])
                tB_ = mkap(tB[:], 0, [[5, cp], [1, 5]])
                nc.vector.tensor_tensor(out=tA_, in0=dg(0), in1=dg(1), op=Alu.add)
                nc.vector.tensor_tensor(out=tB_, in0=dg(2), in1=dg(3), op=Alu.add)
                nc.vector.tensor_tensor(out=tA_, in0=tA_, in1=tB_, op=Alu.add)
                f1 = big.tile([P, CPP, 2], BF16, tag="f1")
                nc.vector.tensor_tensor(out=mkap(f1[:], 0, [[2, cp], [1, 2]]),
                                        in0=mkap(tA[:], 0, [[5, cp], [1, 2]]),
                                        in1=mkap(tA[:], 2, [[5, cp], [1, 2]]),
                                        op=Alu.add)
                ss = big.tile([P, CPP], BF16, tag="ss")
                ss_ = mkap(ss[:], 0, [[1, cp]])
                nc.vector.tensor_tensor(out=ss_,
                                        in0=mkap(f1[:], 0, [[2, cp]]),
                                        in1=mkap(f1[:], 1, [[2, cp]]),
                                        op=Alu.add)
                nc.vector.tensor_tensor(out=ss_, in0=ss_,
                                        in1=mkap(tA[:], 4, [[5, cp]]),
                                        op=Alu.add)

                # masks: obj/noo on Pool (width 1 and 2)
                obj1 = mid.tile([P, CPP], BF16, tag="obj1")
                obj2 = mid.tile([P, CPP, 2], BF16, tag="obj2")
                noo2 = mid.tile([P, CPP, 2], BF16, tag="noo2")
                t4b = ch(tt, 4, [[0, 2]], cp)
                obj1_ = mkap(obj1[:], 0, [[1, cp]])
                obj2_ = mkap(obj2[:], 0, [[1, cp * 2]])
                noo2_ = mkap(noo2[:], 0, [[1, cp * 2]])
                nc.gpsimd.tensor_scalar(out=obj1_, in0=ch(tt, 4, [[1, 1]], cp),
                                        scalar1=0.0, scalar2=None, op0=Alu.is_gt)
                nc.gpsimd.tensor_scalar(out=obj2_, in0=t4b, scalar1=0.0,
                                        scalar2=None, op0=Alu.is_gt)
                nc.gpsimd.tensor_scalar(out=noo2_, in0=t4b, scalar1=0.0,
                                        scalar2=None, op0=Alu.is_le)
                nc.gpsimd.tensor_scalar(out=noo2_, in0=noo2_,
                                        scalar1=SQRT_HALF, scalar2=None,
                                        op0=Alu.mult)

                # conf targets + responsibility masks
                m0, m1 = mkap(m[:], 0, [[2, cp], [1, 1]]), mkap(m[:], 1, [[2, cp], [1, 1]])
                g0, g1 = mkap(g[:], 0, [[2, cp], [1, 1]]), mkap(g[:], 1, [[2, cp], [1, 1]])
                dm = mid.tile([P, CPP, 1], BF16, tag="dm")
                gdm = mid.tile([P, CPP, 1], BF16, tag="gdm")
                ct = mid.tile([P, CPP, 2], BF16, tag="ct")
                dm_ = mkap(dm[:], 0, [[1, cp], [0, 1]])
                gdm_ = mkap(gdm[:], 0, [[1, cp], [0, 1]])
                nc.vector.tensor_tensor(out=dm_, in0=m0, in1=m1, op=Alu.subtract)
                nc.vector.tensor_tensor(out=gdm_, in0=g1, in1=dm_, op=Alu.mult)
                nc.vector.tensor_tensor(out=mkap(ct[:], 0, [[2, cp], [1, 1]]),
                                        in0=m1, in1=gdm_, op=Alu.add)
                nc.vector.tensor_tensor(out=mkap(ct[:], 1, [[2, cp], [1, 1]]),
                                        in0=m0, in1=gdm_, op=Alu.subtract)
                gmin = mid.tile([P, CPP, 1], BF16, tag="gmin")
                rr = mid.tile([P, CPP, 2], BF16, tag="rr")
                gmin_ = mkap(gmin[:], 0, [[1, cp], [0, 1]])
                nc.vector.tensor_tensor(out=gmin_, in0=g0, in1=g1, op=Alu.mult)
                nc.vector.tensor_scalar(out=mkap(rr[:], 0, [[2, cp], [1, 1]]),
                                        in0=gmin_, scalar1=-1.0, scalar2=1.0,
                                        op0=Alu.mult, op1=Alu.add)
                nc.vector.tensor_tensor(out=mkap(rr[:], 1, [[2, cp], [1, 1]]),
                                        in0=g0, in1=g1, op=Alu.max)
                rm = mid.tile([P, CPP, 2], BF16, tag="rm")
                rm5 = mid.tile([P, CPP, 2], BF16, tag="rm5")
                rm5x = mid.tile([P, CPP, 2], BF16, tag="rm5x")
                rm_ = mkap(rm[:], 0, [[1, cp * 2]])
                nc.vector.tensor_tensor(out=rm_, in0=mkap(rr[:], 0, [[1, cp * 2]]),
                                        in1=obj2_, op=Alu.mult)
                rm5_ = mkap(rm5[:], 0, [[1, cp * 2]])
                rm5x_ = mkap(rm5x[:], 0, [[1, cp * 2]])
                nc.vector.tensor_scalar(out=rm5_, in0=rm_, scalar1=SQRT5,
                                        scalar2=None, op0=Alu.mult)
                nc.vector.tensor_scalar(out=rm5x_, in0=rm_,
                                        scalar1=SQRT5 * S / 2.0,
                                        scalar2=None, op0=Alu.mult)

                # class mask by obj + strip writers
                nc.vector.tensor_tensor(out=ss_, in0=ss_, in1=obj1_,
                                        op=Alu.mult)

                # contain: (pc - ct) * rm -> strip[0:2]
                st_e = mkap(strip[:], 0, [[SW, cp], [1, 2]])
                nc.vector.tensor_tensor(out=st_e, in0=ch(pt, 4, [[5, 2]], cp),
                                        in1=mkap(ct[:], 0, [[1, cp * 2]])
                                        .rearrange("p (c k) -> p c k", k=2),
                                        op=Alu.subtract)
                nc.vector.tensor_tensor(out=st_e, in0=st_e,
                                        in1=mkap(rm[:], 0, [[2, cp], [1, 2]]),
                                        op=Alu.mult)

                # noobj: (p49 - t49) * noo*sqrt(.5) -> strip[2:4] (Pool)
                st_n = mkap(strip[:], 2, [[SW, cp], [1, 2]])
                nc.gpsimd.tensor_tensor(out=st_n, in0=ch(pt, 4, [[5, 2]], cp),
                                        in1=ch(tt, 4, [[5, 2]], cp),
                                        op=Alu.subtract)
                nc.gpsimd.tensor_tensor(out=st_n, in0=st_n,
                                        in1=mkap(noo2[:], 0, [[2, cp], [1, 2]]),
                                        op=Alu.mult)

                # loc xy: packed diag |a|*2/7 times rm5x -> strip[4:8] [d][b]
                adiag = mkap(axy[:], 0, [[8, cp], [4, 2], [1, 2]])
                st_xy = mkap(strip[:], 4, [[SW, cp], [2, 2], [1, 2]])
                rm5x_b = mkap(rm5x[:], 0, [[2, cp], [0, 2], [1, 2]])
                nc.vector.tensor_tensor(out=st_xy, in0=adiag, in1=rm5x_b,
                                        op=Alu.mult)

                # loc wh: strip[8:12] -= sqt (contig), times rm5 ([b][d]:
                # rm5 broadcast over d is innermost -> 1x, acceptable)
                st_wh = mkap(strip[:], 8, [[SW, cp], [1, 4]])
                nc.vector.tensor_tensor(out=st_wh, in0=st_wh,
                                        in1=mkap(sqt[:], 0, [[1, cp * 4]]),
                                        op=Alu.subtract)
                st_whbd = mkap(strip[:], 8, [[SW, cp], [2, 2], [1, 2]])
                rm5_b = mkap(rm5[:], 0, [[2, cp], [1, 2], [0, 2]])
                nc.vector.tensor_tensor(out=st_whbd, in0=st_whbd, in1=rm5_b,
                                        op=Alu.mult)

                # defer accumulation into the next iteration's Act queue
                base = rit * NGROUPS
                s_flat = mkap(strip[:], 0, [[1, cp * SW]])
                pending.append((s_flat, ss_, base))

            while pending:
                p_strip, p_ss, p_base = pending.pop()
                nc.scalar.activation(out=p_strip, in_=p_strip, func=Act.Square,
                                     scale=1.0, accum_out=acc[:, p_base:p_base + 1])
                nc.scalar.activation(out=p_ss, in_=p_ss, func=Act.Copy,
                                     scale=1.0, bias=0.0,
                                     accum_out=acc[:, p_base + 1:p_base + 2])

            nc.sync.dma_start(out=out[:], in_=acc[:])

    split_sync_waits(nc)
    return nc


_NC_CACHE = None


def kernel(pred_tensor: np.ndarray, target_tensor: np.ndarray) -> np.ndarray:
    global _NC_CACHE
    if _NC_CACHE is None:
        _NC_CACHE = build_kernel()
    nc = _NC_CACHE

    p = np.ascontiguousarray(pred_tensor, dtype=np.float32).reshape(N_CORES, K_CORE, D)
    t = np.ascontiguousarray(target_tensor, dtype=np.float32).reshape(N_CORES, K_CORE, D)
    in_maps = [{"pred": p[i], "targ": t[i]} for i in range(N_CORES)]
    res = run_bass_kernel_spmd(nc, in_maps, core_ids=list(range(N_CORES)))
    total = 0.0
    for i in range(N_CORES):
        total += res.results[i]["out"].astype(np.float64).sum()
    return np.float32(total / BATCH)


# revision 30
# speedup vs baseline: 4.5287x; 2.2348x over previous
"""YOLO-style loss kernel for Trainium2, 8-core data-parallel.

Strategy: shard the 16384 batch across 8 cores (2048 each = 100352 grid
cells). Each core streams its [cells, 30] fp32 pred/target arrays through
SBUF in 4 wide tiles. Per tile, all five loss terms are reduced to masked
values in one bf16 scratch strip [128, 196, 32] which a single scalar-engine
ACTIVATE(Square, accum_out=...) reduces per tile; term weights (5, 0.5) are
pre-folded into the masks, so each tile needs exactly one accumulate.

The IoU block avoids corner arithmetic: per-axis interval overlap is
  ow = (wp + wt)/2 - max(|cxp - cxt|, |wp - wt|/2),   cx = x/7
which needs only pairwise sums/diffs (i=pred box, j=target box), an Abs on
the scalar engine, one max and one sub on DVE (both 2x bf16). Work is split
across DVE / GpSimd(Pool) / Act so each engine stays under the ~67us DMA
roofline (24 MB/core at ~360 GB/s).

Per-cell math (channels [x0,y0,w0,h0,c0, x1,y1,w1,h1,c1, 20 class]):
  obj  = t4 > 0, noo = t4 == 0
  iou(i,j) from inter = relu(owx)*relu(owy), union = areap_i+areat_j-inter
  g_j = iou(1,j) > iou(0,j), m_j = max_i iou(i,j)
  conf targets ct0 = m1 + g1*(m0-m1), ct1 = m0 - g1*(m0-m1)  (masked wrong
  cases are killed by resp)
  resp_0 = obj*(1-min(g0,g1)), resp_1 = obj*max(g0,g1)
  strip lanes: [0:20] obj*dcls | [20:22] resp*(pc-ct) | [22:24] noo*sqrt(.5)*d49
   | [24:28] resp*sqrt(5)*3.5*(|dxy|*2/7) | [28:32] resp*sqrt(5)*(sqrt(wh+e) diff)
"""

import math

import numpy as np
import concourse.bass as bass
import concourse.tile as tile
from concourse import mybir
from concourse.bass_utils import run_bass_kernel_spmd

F32 = mybir.dt.float32
BF16 = mybir.dt.bfloat16
Alu = mybir.AluOpType
Act = mybir.ActivationFunctionType

# problem constants (hardcoded per harness contract)
BATCH = 16384
S = 7
D = 30
N_CORES = 8
B_PER = BATCH // N_CORES            # 2048
K_CORE = B_PER * S * S              # 100352 cells/core
P = 128
CELLS_PER_PART = K_CORE // P        # 784
NT = 4                              # tiles per core
CPP = CELLS_PER_PART // NT          # 196 cells per partition per tile
EPS = 1e-6
SQRT5 = math.sqrt(5.0)
SQRT_HALF = math.sqrt(0.5)
CLS_POOL = 12                       # class channels masked on Pool (rest DVE)
NGROUPS = 1
RECIP = "hw"                        # "hw" (InstReciprocal) | "newton" (seed+1NR)
U32 = mybir.dt.uint32
RECIP_MAGIC = 0x7EF311C3


def split_sync_waits(nc, max_attached=1):
    """This container's walrus build rejects >1 semaphore wait attached to an
    instruction. Hoist the extras into standalone EventSemaphore wait
    instructions (what raw-bass wait_ge emits), which it accepts."""
    n = 0
    for func in nc.m.functions:
        for bb in func.blocks:
            insts = list(bb.instructions)
            out = []
            changed = False
            for inst in insts:
                si = inst.sync_info
                if si is not None and len(si.on_wait) > max_attached:
                    waits = list(si.on_wait)
                    keep, hoist = waits[:max_attached], waits[max_attached:]
                    for k, w in enumerate(hoist):
                        wi = mybir.InstEventSemaphore(
                            name=f"{inst.name}-hw{k}", ins=[], outs=[]
                        )
                        wi.engine = inst.engine
                        wi.sync_info = mybir.SyncInfo(on_wait=[w], on_update=[])
                        nc.register_instruction(wi, overwrite=True)
                        out.append(wi)
                        n += 1
                    inst.sync_info = mybir.SyncInfo(
                        on_wait=keep, on_update=list(si.on_update)
                    )
                    changed = True
                out.append(inst)
            if changed:
                while len(bb.instructions):
                    bb.instructions.pop()
                for i in out:
                    bb.instructions.append(i)
    return n


def mkap(t_ap, off, dims):
    """AP into a [P, F] tile/view: keep partition dim, custom free dims.
    dims = list of [stride_elems, count]."""
    return bass.AP(tensor=t_ap.tensor, offset=t_ap.offset + off,
                   ap=[list(t_ap.ap[0])] + [list(d) for d in dims])


def ch(t, c0, dims):
    """Box-channel view of an io tile t ([P, CPP*30]): cell-major, channel c0,
    extra dims appended after the cell dim."""
    return mkap(t[:], c0, [[D, CPP]] + dims)


def bc(ap, reps):
    """Replace a trailing singleton dim with a zero-stride broadcast dim."""
    new = [list(d) for d in ap.ap]
    assert new[-1][1] == 1, new
    new[-1] = [0, reps]
    return bass.AP(tensor=ap.tensor, offset=ap.offset, ap=new)


def build_kernel(repeat=1, timing=False):
    nc = bass.Bass("TRN2")
    kind = "Internal" if timing else "ExternalInput"
    pred = nc.dram_tensor("pred", [K_CORE, D], F32, kind=kind)
    targ = nc.dram_tensor("targ", [K_CORE, D], F32, kind=kind)
    NTR = NT * repeat
    out = nc.dram_tensor("out", [P, NTR], F32, kind="ExternalOutput")

    pred_v = pred.ap().rearrange("(n p c) d -> n p (c d)", n=NT, p=P, c=CPP)
    targ_v = targ.ap().rearrange("(n p c) d -> n p (c d)", n=NT, p=P, c=CPP)

    with tile.TileContext(nc) as tc:
        with (
            tc.tile_pool(name="io", bufs=2) as io,
            tc.tile_pool(name="strip", bufs=2) as sp,
            tc.tile_pool(name="mid", bufs=2) as mid,
            tc.tile_pool(name="mid1", bufs=2) as mid1,
            tc.tile_pool(name="accp", bufs=1) as accp,
        ):
            acc = accp.tile([P, NTR], F32)
            eps_t = accp.tile([P, 1], F32)
            zero_t = accp.tile([P, 1], F32)
            nc.vector.memset(eps_t[:], EPS)
            nc.vector.memset(zero_t[:], 0.0)

            for rit in range(NTR):
                it = rit % NT
                pt = io.tile([P, CPP * D], F32, tag="pt")
                tt = io.tile([P, CPP * D], F32, tag="tt")
                nc.sync.dma_start(out=pt[:], in_=pred_v[it])
                nc.sync.dma_start(out=tt[:], in_=targ_v[it])

                strip = sp.tile([P, CPP, 32], BF16, tag="strip")

                # ---- pairwise sums and diffs [P, C, 2d, 2j, 2i] ----
                # pred channel dep: i only; targ: j only. d: w/h (or x/y).
                sxy = mid.tile([P, CPP, 8], BF16, tag="sxy")
                aaw = mid1.tile([P, CPP, 16], BF16, tag="aaw")
                # ISA allows max 3 free dims: one instr per d (x/y, w/h)
                for dd in range(2):
                    s_out = mkap(sxy[:], 4 * dd, [[8, CPP], [2, 2], [1, 2]])
                    nc.vector.tensor_tensor(
                        out=s_out,
                        in0=ch(pt, 2 + dd, [[0, 2], [5, 2]]),
                        in1=ch(tt, 2 + dd, [[5, 2], [0, 2]]), op=Alu.add)
                    a_out = mkap(aaw[:], 4 * dd, [[16, CPP], [2, 2], [1, 2]])
                    nc.vector.tensor_tensor(
                        out=a_out,
                        in0=ch(pt, 0 + dd, [[0, 2], [5, 2]]),
                        in1=ch(tt, 0 + dd, [[5, 2], [0, 2]]), op=Alu.subtract)
                    w_out = mkap(aaw[:], 8 + 4 * dd, [[16, CPP], [2, 2], [1, 2]])
                    nc.vector.tensor_tensor(
                        out=w_out,
                        in0=ch(pt, 2 + dd, [[0, 2], [5, 2]]),
                        in1=ch(tt, 2 + dd, [[5, 2], [0, 2]]), op=Alu.subtract)

                # |a|*2/7 and |dw| in place (Act)
                aa = mkap(aaw[:], 0, [[16, CPP], [1, 8]])
                ww = mkap(aaw[:], 8, [[16, CPP], [1, 8]])
                nc.scalar.activation(out=aa, in_=aa, func=Act.Abs,
                                     bias=zero_t[:], scale=2.0 / S)
                nc.scalar.activation(out=ww, in_=ww, func=Act.Abs,
                                     bias=zero_t[:], scale=1.0)

                # hm = max(|a|2/7, |dw|); q = s - hm (both 2x bf16)
                hm = mid.tile([P, CPP, 8], BF16, tag="hm")
                nc.vector.tensor_tensor(out=hm[:], in0=aa, in1=ww, op=Alu.max)
                nc.vector.tensor_tensor(out=sxy[:], in0=sxy[:], in1=hm[:],
                                        op=Alu.subtract)
                # oc = relu(0.5*q) (Act)
                oc = mid.tile([P, CPP, 8], BF16, tag="oc")
                nc.scalar.activation(out=oc[:], in_=sxy[:], func=Act.Relu,
                                     bias=zero_t[:], scale=0.5)

                # inter[j,i] = ow_x * ow_y (2x)
                inter = mid.tile([P, CPP, 4], BF16, tag="inter")
                o_x = mkap(oc[:], 0, [[8, CPP], [2, 2], [1, 2]])
                o_y = mkap(oc[:], 4, [[8, CPP], [2, 2], [1, 2]])
                nc.vector.tensor_tensor(out=inter[:], in0=o_x, in1=o_y,
                                        op=Alu.mult)

                # areas + pairwise union (Pool), then 1/union (DVE)
                areap = mid.tile([P, CPP, 2], F32, tag="areap")
                areat = mid.tile([P, CPP, 2], F32, tag="areat")
                nc.gpsimd.tensor_tensor(out=areap[:], in0=ch(pt, 2, [[5, 2]]),
                                        in1=ch(pt, 3, [[5, 2]]), op=Alu.mult)
                nc.gpsimd.tensor_tensor(out=areat[:], in0=ch(tt, 2, [[5, 2]]),
                                        in1=ch(tt, 3, [[5, 2]]), op=Alu.mult)
                uni = mid.tile([P, CPP, 2, 2], F32, tag="uni")
                ap_b = mkap(areap[:], 0, [[2, CPP], [0, 2], [1, 2]])
                at_b = mkap(areat[:], 0, [[2, CPP], [1, 2], [0, 2]])
                nc.gpsimd.tensor_tensor(out=uni[:], in0=ap_b, in1=at_b,
                                        op=Alu.add)
                uni4 = uni[:].rearrange("p c a b -> p (c a b)")
                inter4 = inter[:].rearrange("p c k -> p (c k)")
                nc.vector.tensor_tensor(out=uni4, in0=uni4, in1=inter4,
                                        op=Alu.subtract)
                run = mid.tile([P, CPP, 4], F32, tag="run")
                run4 = run[:].rearrange("p c k -> p (c k)")
                if RECIP == "hw":
                    nc.vector.reciprocal(out=run4, in_=uni4)
                else:
                    # seed via magic-constant exponent flip, then 1 Newton step
                    # magic - u == (u ^ 0xFFFFFFFF) + (magic+1)  (mod 2^32)
                    nc.vector.tensor_scalar(
                        out=run4.bitcast(U32), in0=uni4.bitcast(U32),
                        scalar1=0xFFFFFFFF, scalar2=None, op0=Alu.bitwise_xor)
                    nc.vector.tensor_scalar(
                        out=run4.bitcast(U32), in0=run4.bitcast(U32),
                        scalar1=RECIP_MAGIC + 1, scalar2=None, op0=Alu.add)
                    rtmp = mid.tile([P, CPP * 4], F32, tag="rtmp")
                    nc.vector.tensor_tensor(out=rtmp[:], in0=uni4, in1=run4,
                                            op=Alu.mult)
                    nc.vector.tensor_scalar(out=rtmp[:], in0=rtmp[:],
                                            scalar1=-1.0, scalar2=2.0,
                                            op0=Alu.mult, op1=Alu.add)
                    nc.vector.tensor_tensor(out=run4, in0=run4, in1=rtmp[:],
                                            op=Alu.mult)

                # iou, transposed to [i][j] so m/g run 2x over packed j
                iou = mid.tile([P, CPP, 2, 2], BF16, tag="iou")  # [i][j]
                iou_t = mkap(iou[:], 0, [[4, CPP], [1, 2], [2, 2]])  # [c][j][i]
                nc.vector.tensor_tensor(out=iou_t, in0=inter[:], in1=run[:],
                                        op=Alu.mult)
                m = mid.tile([P, CPP, 2], BF16, tag="m")
                g = mid.tile([P, CPP, 2], BF16, tag="g")
                iou_i0 = mkap(iou[:], 0, [[4, CPP], [1, 2]])
                iou_i1 = mkap(iou[:], 2, [[4, CPP], [1, 2]])
                nc.vector.tensor_tensor(out=m[:], in0=iou_i0, in1=iou_i1,
                                        op=Alu.max)
                nc.vector.tensor_tensor(out=g[:], in0=iou_i1, in1=iou_i0,
                                        op=Alu.is_gt)

                # conf targets + responsibility masks (Pool)
                m0, m1 = m[:, :, 0:1], m[:, :, 1:2]
                g0, g1 = g[:, :, 0:1], g[:, :, 1:2]
                dm = mid.tile([P, CPP, 1], BF16, tag="dm")
                gdm = mid.tile([P, CPP, 1], BF16, tag="gdm")
                ct = mid.tile([P, CPP, 2], BF16, tag="ct")
                nc.gpsimd.tensor_tensor(out=dm[:], in0=m0, in1=m1, op=Alu.subtract)
                nc.gpsimd.tensor_tensor(out=gdm[:], in0=g1, in1=dm[:], op=Alu.mult)
                nc.gpsimd.tensor_tensor(out=ct[:, :, 0:1], in0=m1, in1=gdm[:], op=Alu.add)
                nc.gpsimd.tensor_tensor(out=ct[:, :, 1:2], in0=m0, in1=gdm[:], op=Alu.subtract)

                obj = mid.tile([P, CPP, 1], BF16, tag="obj")
                noo = mid.tile([P, CPP, 1], BF16, tag="noo")
                t4 = ch(tt, 4, [[1, 1]])
                nc.gpsimd.tensor_scalar(out=obj[:], in0=t4, scalar1=0.0,
                                        scalar2=None, op0=Alu.is_gt)
                nc.gpsimd.tensor_scalar(out=noo[:], in0=t4, scalar1=0.0,
                                        scalar2=None, op0=Alu.is_le)
                nc.gpsimd.tensor_scalar(out=noo[:], in0=noo[:], scalar1=SQRT_HALF,
                                        scalar2=None, op0=Alu.mult)
                gmin = mid.tile([P, CPP, 1], BF16, tag="gmin")
                rr = mid.tile([P, CPP, 2], BF16, tag="rr")
                # g binary: min(g0,g1) == g0*g1 (Pool tt: add/sub/mult only)
                nc.gpsimd.tensor_tensor(out=gmin[:], in0=g0, in1=g1, op=Alu.mult)
                nc.gpsimd.tensor_scalar(out=rr[:, :, 0:1], in0=gmin[:],
                                        scalar1=-1.0, scalar2=1.0,
                                        op0=Alu.mult, op1=Alu.add)
                nc.vector.tensor_tensor(out=rr[:, :, 1:2], in0=g0, in1=g1, op=Alu.max)
                rm = mid.tile([P, CPP, 2], BF16, tag="rm")
                rm5 = mid.tile([P, CPP, 2], BF16, tag="rm5")
                rm5x = mid.tile([P, CPP, 2], BF16, tag="rm5x")
                nc.gpsimd.tensor_tensor(out=rm[:], in0=rr[:], in1=bc(obj[:], 2),
                                        op=Alu.mult)
                nc.gpsimd.tensor_scalar(out=rm5[:], in0=rm[:], scalar1=SQRT5,
                                        scalar2=None, op0=Alu.mult)
                nc.gpsimd.tensor_scalar(out=rm5x[:], in0=rm[:], scalar1=SQRT5 * S / 2.0,
                                        scalar2=None, op0=Alu.mult)

                # contain: (pc - ct) * rm -> strip[20:22]
                st_e = mkap(strip[:], 20, [[32, CPP], [1, 2]])
                nc.vector.tensor_tensor(out=st_e, in0=ch(pt, 4, [[5, 2]]),
                                        in1=ct[:], op=Alu.subtract)
                nc.vector.tensor_tensor(out=st_e, in0=st_e, in1=rm[:], op=Alu.mult)

                # noobj: (p49 - t49) * noo*sqrt(.5) -> strip[22:24] (Pool)
                st_n = mkap(strip[:], 22, [[32, CPP], [1, 2]])
                nc.gpsimd.tensor_tensor(out=st_n, in0=ch(pt, 4, [[5, 2]]),
                                        in1=ch(tt, 4, [[5, 2]]), op=Alu.subtract)
                nc.gpsimd.tensor_tensor(out=st_n, in0=st_n,
                                        in1=bc(noo[:], 2), op=Alu.mult)

                # loc xy: diag(|a|*2/7) * rm5x -> strip[24:28]  [d][b]
                adiag = mkap(aaw[:], 0, [[16, CPP], [4, 2], [3, 2]])
                st_xy = mkap(strip[:], 24, [[32, CPP], [2, 2], [1, 2]])
                rm5x_b = mkap(rm5x[:], 0, [[2, CPP], [0, 2], [1, 2]])
                nc.vector.tensor_tensor(out=st_xy, in0=adiag, in1=rm5x_b,
                                        op=Alu.mult)

                # loc wh: (sqrt(p_wh+eps)-sqrt(t_wh+eps)) * rm5 -> strip[28:32]
                sqp = mid.tile([P, CPP, 2, 2], BF16, tag="sqp")  # [b][d]
                sqt = mid.tile([P, CPP, 2, 2], BF16, tag="sqt")
                nc.scalar.activation(out=sqp[:], in_=ch(pt, 2, [[5, 2], [1, 2]]),
                                     func=Act.Sqrt, bias=eps_t[:], scale=1.0)
                nc.scalar.activation(out=sqt[:], in_=ch(tt, 2, [[5, 2], [1, 2]]),
                                     func=Act.Sqrt, bias=eps_t[:], scale=1.0)
                st_wh = mkap(strip[:], 28, [[32, CPP], [1, 4]])
                nc.vector.tensor_tensor(
                    out=st_wh, in0=sqp[:].rearrange("p c a b -> p c (a b)"),
                    in1=sqt[:].rearrange("p c a b -> p c (a b)"), op=Alu.subtract)
                st_wh2 = mkap(strip[:], 28, [[32, CPP], [2, 2], [1, 2]])
                rm5_b = mkap(rm5[:], 0, [[2, CPP], [1, 2], [0, 2]])
                nc.vector.tensor_tensor(out=st_wh2, in0=st_wh2, in1=rm5_b,
                                        op=Alu.mult)

                # class: dcls -> strip[0:20] (DVE), obj-mask split Pool/DVE
                st_c = mkap(strip[:], 0, [[32, CPP], [1, 20]])
                nc.vector.tensor_tensor(out=st_c, in0=ch(pt, 10, [[1, 20]]),
                                        in1=ch(tt, 10, [[1, 20]]), op=Alu.subtract)
                kp = CLS_POOL
                st_cp = mkap(strip[:], 0, [[32, CPP], [1, kp]])
                nc.gpsimd.tensor_tensor(out=st_cp, in0=st_cp,
                                        in1=bc(obj[:], kp), op=Alu.mult)
                st_cv = mkap(strip[:], kp, [[32, CPP], [1, 20 - kp]])
                nc.vector.tensor_tensor(out=st_cv, in0=st_cv,
                                        in1=bc(obj[:], 20 - kp), op=Alu.mult)

                # single fused square+accumulate for the whole strip
                s_flat = strip[:].rearrange("p c w -> p (c w)")
                nc.scalar.activation(out=s_flat, in_=s_flat, func=Act.Square,
                                     scale=1.0, accum_out=acc[:, rit:rit + 1])

            nc.sync.dma_start(out=out[:], in_=acc[:])

    split_sync_waits(nc)
    return nc


_NC_CACHE = None


def kernel(pred_tensor: np.ndarray, target_tensor: np.ndarray) -> np.ndarray:
    global _NC_CACHE
    if _NC_CACHE is None:
        _NC_CACHE = build_kernel()
    nc = _NC_CACHE

    p = np.ascontiguousarray(pred_tensor, dtype=np.float32).reshape(N_CORES, K_CORE, D)
    t = np.ascontiguousarray(target_tensor, dtype=np.float32).reshape(N_CORES, K_CORE, D)
    in_maps = [{"pred": p[i], "targ": t[i]} for i in range(N_CORES)]
    res = run_bass_kernel_spmd(nc, in_maps, core_ids=list(range(N_CORES)))
    total = 0.0
    for i in range(N_CORES):
        total += res.results[i]["out"].astype(np.float64).sum()
    return np.float32(total / BATCH)
